# revision 4
# baseline (speedup 1.0000x reference)
# Trainium2 Bass kernel for nn_TemporalGCN (LSTM -> 2x GCN -> pairwise edge MLP).
#
# Sharding: pure data-parallel over B (8 batch elements -> 8 NeuronCores).
# Each core runs an identical program on its own batch element; no collectives.
#
# Key algebraic simplification: the GCN message pass
#   messages = einsum('ij,ijn->in', adj, edge @ epW.T + epb)
# collapses to   wedge @ epW.T + rowsum(adj) x epb   with
#   wedge[i,e] = sum_j adj[i,j] * edge[i,j,e]   ([N, E] only).
# Only edge_features[:, -1] is ever used, so the big [B,W,N,N,E] tensor is
# sliced on the host and never shipped.

import numpy as np

import concourse.bass as bass
import concourse.bacc as bacc
import concourse.tile as tile
from concourse import mybir
from concourse import bass_utils

H, E, F, B, W, N = 64, 5, 6, 8, 12, 200
LN_EPS = 1e-5

F32 = mybir.dt.float32
BF16 = mybir.dt.bfloat16
NPBF = mybir.dt.np(BF16)
AF = mybir.ActivationFunctionType
ALU = mybir.AluOpType
AX = mybir.AxisListType

CHUNKS = [(0, 128), (128, 72)]      # destination-node chunks over i
NTILE_MLP = 25                       # edge-mlp pair tiles (4 j-pairs each)
NGROUP = 13                          # 16-j output groups (last has 8 j)

# ---------------------------------------------------------------- blob layout
class _Cols:
    def __init__(self):
        self.c = 0
        self.slabs = {}

    def add(self, name, width):
        self.slabs[name] = (self.c, width)
        self.c += width

WB = _Cols()
WB.add("lhsT_x0", 256)     # 4 slots of Wih0.T at rows 32s..32s+6
WB.add("lhsT_h0", 256)     # rows 64:128 = Whh0.T
WB.add("lhsT_l1", 256)     # rows 0:64 = Whh1p.T, rows 64:128 = Wih1p.T
WB.add("identB", 128)      # bf16 identity
WB.add("lhsT_W1hi", 128)   # blockdiag(W1hi.T, W1hi.T)
WB.add("lhsT_e", 128)      # 4 slots of blockdiag(W1e.T, W1e.T) at rows 32s..32s+10
WB.add("lhsT_W2", 64)      # blockdiag(W2.T, W2.T)
WB.add("lhsT_W3", 4)       # col q rows 32q:32q+32 = w3
WB.add("W1hjT", 64)        # rows 0:64
WB.add("rhs_ep0", 64)      # rows 0:5 = epW0.T, row 5 = epb0
WB.add("rhs_ep1", 64)
WB.add("rhs_gcnW0", 64)    # rows 0:64
WB.add("rhs_gcnW1", 64)
WB.add("rhs_gcnb0", 64)    # row 0
WB.add("rhs_gcnb1", 64)
WB.add("ones_row", 200)    # row 0 = 1.0

WF = _Cols()
WF.add("biasL", 4)         # LSTM biases: l0c0, l0c1, l1c0, l1c1
WF.add("identF", 128)      # f32 identity
WF.add("lhsT_I2", 64)      # [I64; I64] f32 (cross-half pair sum)
WF.add("b1col", 1)         # rows 0:64 = mlp_b1
WF.add("b2x4", 1)          # rows 0:128 = tile(mlp_b2, 4)
WF.add("b3col", 1)         # rows 0:128 = mlp_b3 (sigmoid bias)
WF.add("ln_g0", 64)        # row 0 (used via partition-broadcast DMA)
WF.add("ln_b0", 64)
WF.add("ln_g1", 64)
WF.add("ln_b1", 64)


def _pack_weights(inp):
    wb = np.zeros((128, WB.c), np.float32)
    wf = np.zeros((128, WF.c), np.float32)

    def put_b(name, rows, arr):
        c0, w = WB.slabs[name]
        arr = np.asarray(arr, np.float32)
        wb[rows, c0:c0 + arr.shape[1]] = arr

    def put_f(name, rows, arr):
        c0, w = WF.slabs[name]
        arr = np.asarray(arr, np.float32)
        wf[rows, c0:c0 + arr.shape[1]] = arr

    Wih0, Whh0 = inp["Wih0"], inp["Whh0"]
    b0 = inp["bih0"] + inp["bhh0"]
    # layer1 gate permutation: chunk0 = [f, i], chunk1 = [o, g]
    perm1 = np.concatenate([np.arange(64, 128), np.arange(0, 64),
                            np.arange(192, 256), np.arange(128, 192)])
    Wih1p = inp["Wih1"][perm1]
    Whh1p = inp["Whh1"][perm1]
    b1p = (inp["bih1"] + inp["bhh1"])[perm1]

    for s in range(4):
        put_b("lhsT_x0", slice(32 * s, 32 * s + 6), Wih0.T)
    put_b("lhsT_h0", slice(64, 128), Whh0.T)
    put_b("lhsT_l1", slice(0, 64), Whh1p.T)
    put_b("lhsT_l1", slice(64, 128), Wih1p.T)
    put_b("identB", slice(0, 128), np.eye(128))

    W1 = inp["mlp_W1"]                      # [64, 133]
    W1hi, W1hj, W1e = W1[:, :64], W1[:, 64:128], W1[:, 128:133]
    bd = np.zeros((128, 128), np.float32)
    bd[0:64, 0:64] = W1hi.T
    bd[64:128, 64:128] = W1hi.T
    put_b("lhsT_W1hi", slice(0, 128), bd)
    ebd = np.zeros((128, 128), np.float32)
    for s in range(4):
        ebd[32 * s:32 * s + 5, 0:64] = W1e.T
        ebd[32 * s + 5:32 * s + 10, 64:128] = W1e.T
    put_b("lhsT_e", slice(0, 128), ebd)
    W2 = inp["mlp_W2"]                      # [32, 64]
    w2bd = np.zeros((128, 64), np.float32)
    w2bd[0:64, 0:32] = W2.T
    w2bd[64:128, 32:64] = W2.T
    put_b("lhsT_W2", slice(0, 128), w2bd)
    w3 = np.zeros((128, 4), np.float32)
    for q in range(4):
        w3[32 * q:32 * q + 32, q] = inp["mlp_W3"][0]
    put_b("lhsT_W3", slice(0, 128), w3)
    put_b("W1hjT", slice(0, 64), W1hj.T)

    for l in range(2):
        ep = np.zeros((6, 64), np.float32)
        ep[0:5] = inp["ep_W"][l].T
        ep[5] = inp["ep_b"][l]
        put_b(f"rhs_ep{l}", slice(0, 6), ep)
        put_b(f"rhs_gcnW{l}", slice(0, 64), inp["gcn_W"][l].T)
        put_b(f"rhs_gcnb{l}", slice(0, 1), inp["gcn_b"][l][None, :])
    put_b("ones_row", slice(0, 1), np.ones((1, 200)))

    bias_cols = np.stack([b0[0:128], b0[128:256], b1p[0:128], b1p[128:256]], axis=1)
    put_f("biasL", slice(0, 128), bias_cols)
    put_f("identF", slice(0, 128), np.eye(128))
    i2 = np.concatenate([np.eye(64), np.eye(64)], axis=0)
    put_f("lhsT_I2", slice(0, 128), i2)
    put_f("b1col", slice(0, 64), inp["mlp_b1"][:, None])
    put_f("b2x4", slice(0, 128), np.tile(inp["mlp_b2"], 4)[:, None])
    put_f("b3col", slice(0, 128), np.full((128, 1), float(np.asarray(inp["mlp_b3"]).reshape(-1)[0]), np.float32))
    for l in range(2):
        put_f(f"ln_g{l}", slice(0, 1), inp["ln_g"][l][None, :])
        put_f(f"ln_b{l}", slice(0, 1), inp["ln_b"][l][None, :])

    return wb.astype(NPBF), wf


def _pack_core(inp, b):
    """Per-core (per-batch-element) data blobs."""
    edge = np.asarray(inp["edge_features"][b, -1], np.float32)   # [N, N, E]
    adj = np.asarray(inp["adjacency"][b], np.float32)            # [N, N]
    node = np.asarray(inp["node_features"][b], np.float32)       # [W, N, F]

    # xpad [128, 600]: tile k, slot s, rows 32s:32s+6 = node[4k+s].T
    xt = node.transpose(0, 2, 1)                                  # [W, F, N]
    xp = np.zeros((3, 4, 32, 200), np.float32)
    xp[:, :, 0:6] = xt.reshape(3, 4, 6, 200)
    xpad_full = np.zeros((128, 600), np.float32)
    for s in range(4):
        for k in range(3):
            xpad_full[32 * s:32 * s + 32, 200 * k:200 * k + 200] = xp[k, s]

    # edge_w [128, 2000]: (i, e-major*j) layout, chunk1 in cols 1000:2000
    ew = edge.transpose(0, 2, 1).reshape(200, 1000)               # [i, e*200+j]
    edge_w = np.zeros((128, 2000), np.float32)
    edge_w[:, 0:1000] = ew[0:128]
    edge_w[0:72, 1000:2000] = ew[128:200]

    # edge_mlp [128, 5000]: tile m holds pairs 4m..4m+3; pair rows = 2 j's x 5 e
    em = edge.transpose(1, 2, 0)                                  # [j, e, i]
    edge_mlp = np.zeros((128, 5000), np.float32)
    for m in range(25):
        for p in range(4):
            edge_mlp[32 * p:32 * p + 10, 200 * m:200 * m + 200] = \
                em[8 * m + 2 * p:8 * m + 2 * p + 2].reshape(10, 200)

    return {
        "xpad": xpad_full.astype(NPBF),
        "edge_w": edge_w.astype(NPBF),
        "edge_mlp": edge_mlp.astype(NPBF),
        "adj": adj.astype(NPBF),
    }


# ---------------------------------------------------------------- bass program
def _build(debug=False):
    nc = bacc.Bacc("TRN2", target_bir_lowering=False)
    d = {}
    d["xpad"] = nc.dram_tensor("xpad", [128, 600], BF16, kind="ExternalInput").ap()
    d["edge_w"] = nc.dram_tensor("edge_w", [128, 2000], BF16, kind="ExternalInput").ap()
    d["edge_mlp"] = nc.dram_tensor("edge_mlp", [128, 5000], BF16, kind="ExternalInput").ap()
    d["adj"] = nc.dram_tensor("adj", [200, 200], BF16, kind="ExternalInput").ap()
    d["wb16"] = nc.dram_tensor("wb16", [128, WB.c], BF16, kind="ExternalInput").ap()
    d["wf32"] = nc.dram_tensor("wf32", [128, WF.c], F32, kind="ExternalInput").ap()
    d_out = nc.dram_tensor("outT", [200, 200], F32, kind="ExternalOutput").ap()
    dbg = {}
    if debug:
        for nm, shp, dt in [("dbg_h", [64, 200], BF16), ("dbg_h1", [64, 200], BF16),
                            ("dbg_h2", [64, 200], BF16), ("dbg_w6", [200, 6], F32),
                            ("dbg_cb", [64, 200], F32)]:
            dbg[nm] = nc.dram_tensor(nm, shp, dt, kind="ExternalOutput").ap()

    with tile.TileContext(nc) as tc:
        _body(nc, tc, d, d_out, dbg)
    nc.compile()
    return nc


def _body(nc, tc, d, d_out, dbg):
    import contextlib
    ctx = contextlib.ExitStack()
    with ctx:
        consts = ctx.enter_context(tc.tile_pool(name="consts", bufs=1))
        work = ctx.enter_context(tc.tile_pool(name="work", bufs=3))

        # ---------------- load constants + inputs
        wb = consts.tile([128, WB.c], BF16)
        nc.sync.dma_start(out=wb, in_=d["wb16"])
        wf = consts.tile([128, WF.c], F32)
        nc.sync.dma_start(out=wf, in_=d["wf32"])

        def WBS(name, rows=slice(0, 128)):
            c0, w = WB.slabs[name]
            return wb[rows, c0:c0 + w]

        def WFS(name, rows=slice(0, 128)):
            c0, w = WF.slabs[name]
            return wf[rows, c0:c0 + w]

        xpad = consts.tile([128, 600], BF16)
        nc.sync.dma_start(out=xpad, in_=d["xpad"])
        edge_w = consts.tile([128, 2000], BF16)
        nc.sync.dma_start(out=edge_w, in_=d["edge_w"])
        edge_mlp = consts.tile([128, 5000], BF16)
        nc.sync.dma_start(out=edge_mlp, in_=d["edge_mlp"])

        # adj broadcast x5 (partition rows = i, free = (e, j) with e step 0)
        adjx = []
        for k, (i0, ck) in enumerate(CHUNKS):
            t = consts.tile([128, 5, 200], BF16, tag=f"adjx{k}")
            src = bass.AP(tensor=d["adj"].tensor, offset=i0 * 200,
                          ap=[[200, ck], [0, 5], [1, 200]])
            nc.sync.dma_start(out=t[0:ck], in_=src)
            adjx.append(t)

        # LN scale/bias broadcast tiles
        lng, lnb = [], []
        for l in range(2):
            g = consts.tile([128, 64], F32, tag=f"lng{l}")
            bt = consts.tile([128, 64], F32, tag=f"lnb{l}")
            for t, nm in ((g, f"ln_g{l}"), (bt, f"ln_b{l}")):
                c0, wdt = WF.slabs[nm]
                src = bass.AP(tensor=d["wf32"].tensor, offset=c0, ap=[[0, 128], [1, 64]])
                nc.sync.dma_start(out=t, in_=src)
            lng.append(g)
            lnb.append(bt)

        eps_t = consts.tile([128, 1], F32)
        nc.vector.memset(eps_t, LN_EPS)

        # ---------------- wedge: wedge[i, e] = sum_j adj[i,j]*edge[i,j,e]
        w6 = []
        for k, (i0, ck) in enumerate(CHUNKS):
            tmp = work.tile([128, 1000], BF16, tag="wtmp")
            nc.gpsimd.tensor_mul(
                out=tmp[0:ck],
                in0=edge_w[0:ck, 1000 * k:1000 * k + 1000],
                in1=adjx[k][0:ck].rearrange("p e j -> p (e j)"),
            )
            w6c = consts.tile([128, 6], F32, tag=f"w6_{k}")
            nc.vector.reduce_sum(out=w6c[0:ck, 0:5],
                                 in_=tmp[0:ck].rearrange("p (e j) -> p e j", j=200),
                                 axis=AX.X)
            nc.vector.reduce_sum(out=w6c[0:ck, 5:6],
                                 in_=adjx[k][0:ck, 0:1, :], axis=AX.X)
            w6.append(w6c)
        if dbg:
            nc.gpsimd.dma_start(out=dbg["dbg_w6"][0:128, :], in_=w6[0][0:128])
            nc.gpsimd.dma_start(out=dbg["dbg_w6"][128:200, :], in_=w6[1][0:72])

        # ---------------- LSTM (12 steps x 2 layers)
        hTfin = consts.tile([128, 200], BF16, tag="hTfin")
        with tc.tile_pool(name="ps_g", bufs=4, space="PSUM") as ps_g, \
             tc.tile_pool(name="ps_c", bufs=2, space="PSUM") as ps_c, \
             tc.tile_pool(name="lstm", bufs=4) as lp:
            Y = [lp.tile([128, 200], BF16, tag="Y", name=f"Y{t}") for t in range(12)] + [hTfin]
            c_prev = [None, None]
            for t in range(12):
                s = t % 4
                k = t // 4
                rhs_x = xpad[32 * s:32 * s + 6, 200 * k:200 * k + 200]
                # ---- layer 0: chunk0 = [i; f], chunk1 = [g; o]
                P = []
                for c in range(2):
                    pg = ps_g.tile([128, 200], F32, tag="g")
                    nc.tensor.matmul(pg, WBS("lhsT_x0", slice(32 * s, 32 * s + 6))[:, 128 * c:128 * c + 128],
                                     rhs_x, start=True, stop=(t == 0),
                                     tile_position=(32 * s, 0))
                    if t > 0:
                        nc.tensor.matmul(pg, WBS("lhsT_h0", slice(64, 128))[:, 128 * c:128 * c + 128],
                                         Y[t - 1][64:128, :], start=False, stop=True)
                    P.append(pg)
                S0 = lp.tile([128, 200], F32, tag="S")
                nc.scalar.activation(out=S0, in_=P[0], func=AF.Sigmoid, bias=WFS("biasL")[:, 0:1])
                TGO = lp.tile([128, 200], F32, tag="T")
                nc.scalar.activation(out=TGO[0:64], in_=P[1][0:64], func=AF.Tanh,
                                     bias=WFS("biasL", slice(0, 64))[:, 1:2])
                nc.scalar.activation(out=TGO[64:128], in_=P[1][64:128], func=AF.Sigmoid,
                                     bias=WFS("biasL", slice(64, 128))[:, 1:2])
                M0 = lp.tile([128, 200], F32, tag="M")
                nc.vector.tensor_mul(out=M0[0:64], in0=S0[0:64], in1=TGO[0:64])
                if t > 0:
                    nc.vector.tensor_mul(out=M0[64:128], in0=S0[64:128], in1=c_prev[0][64:128])
                else:
                    nc.vector.memset(M0[64:128], 0.0)
                c0n = ps_c.tile([128, 200], F32, tag="c0")
                nc.tensor.matmul(c0n[64:128], WFS("lhsT_I2"), M0, start=True, stop=True)
                TC0 = lp.tile([128, 200], F32, tag="TC")
                nc.scalar.activation(out=TC0[64:128], in_=c0n[64:128], func=AF.Tanh)
                nc.vector.tensor_mul(out=Y[t][64:128], in0=TGO[64:128], in1=TC0[64:128])
                c_prev[0] = c0n
                # ---- layer 1: chunk0 = [f; i], chunk1 = [o; g]
                P1 = []
                for c in range(2):
                    pg = ps_g.tile([128, 200], F32, tag="g")
                    if t == 0:
                        nc.tensor.matmul(pg, WBS("lhsT_l1", slice(64, 128))[:, 128 * c:128 * c + 128],
                                         Y[t][64:128, :], start=True, stop=True)
                    else:
                        nc.tensor.matmul(pg, WBS("lhsT_l1")[:, 128 * c:128 * c + 128],
                                         Y[t], start=True, stop=True)
                    P1.append(pg)
                S1 = lp.tile([128, 200], F32, tag="S")
                nc.scalar.activation(out=S1, in_=P1[0], func=AF.Sigmoid, bias=WFS("biasL")[:, 2:3])
                OG = lp.tile([128, 200], F32, tag="T")
                nc.scalar.activation(out=OG[0:64], in_=P1[1][0:64], func=AF.Sigmoid,
                                     bias=WFS("biasL", slice(0, 64))[:, 3:4])
                nc.scalar.activation(out=OG[64:128], in_=P1[1][64:128], func=AF.Tanh,
                                     bias=WFS("biasL", slice(64, 128))[:, 3:4])
                M1 = lp.tile([128, 200], F32, tag="M")
                if t > 0:
                    nc.vector.tensor_mul(out=M1[0:64], in0=S1[0:64], in1=c_prev[1][0:64])
                else:
                    nc.vector.memset(M1[0:64], 0.0)
                nc.vector.tensor_mul(out=M1[64:128], in0=S1[64:128], in1=OG[64:128])
                c1n = ps_c.tile([128, 200], F32, tag="c1")
                nc.tensor.matmul(c1n[0:64], WFS("lhsT_I2"), M1, start=True, stop=True)
                TC1 = lp.tile([128, 200], F32, tag="TC")
                nc.scalar.activation(out=TC1[0:64], in_=c1n[0:64], func=AF.Tanh)
                nc.vector.tensor_mul(out=Y[t + 1][0:64], in0=OG[0:64], in1=TC1[0:64])
                c_prev[1] = c1n
        if dbg:
            nc.gpsimd.dma_start(out=dbg["dbg_h"], in_=hTfin[0:64])

        # ---------------- wedge transpose ([i,6] chunks -> wedgeT6 [6, 200])
        wedgeT6 = consts.tile([6, 200], BF16)
        with tc.tile_pool(name="ps_wt", bufs=2, space="PSUM") as ps_wt:
            for k, (i0, ck) in enumerate(CHUNKS):
                pwt = ps_wt.tile([6, 128], F32, tag="wt")
                nc.tensor.transpose(pwt[:, 0:ck], w6[k][0:ck], WFS("identF", slice(0, ck))[:, 0:ck])
                nc.vector.tensor_copy(out=wedgeT6[:, i0:i0 + ck], in_=pwt[:, 0:ck])

        # ---------------- GCN (2 layers)
        hT_cur = hTfin
        with tc.tile_pool(name="ps_u", bufs=2, space="PSUM") as ps_u, \
             tc.tile_pool(name="ps_t", bufs=2, space="PSUM") as ps_t:
            for l in range(2):
                hT_next = consts.tile([64, 200], BF16, tag=f"hT{l + 1}")
                for k, (i0, ck) in enumerate(CHUNKS):
                    pu = ps_u.tile([128, 64], F32, tag="u")
                    nc.tensor.matmul(pu[0:ck], wedgeT6[:, i0:i0 + ck], WBS(f"rhs_ep{l}", slice(0, 6)),
                                     start=True, stop=False)
                    nc.tensor.matmul(pu[0:ck], hT_cur[0:64, i0:i0 + ck], WBS(f"rhs_gcnW{l}", slice(0, 64)),
                                     start=False, stop=False)
                    nc.tensor.matmul(pu[0:ck], WBS("ones_row", slice(0, 1))[:, i0:i0 + ck],
                                     WBS(f"rhs_gcnb{l}", slice(0, 1)), start=False, stop=True)
                    stats = work.tile([128, nc.vector.BN_STATS_DIM], F32, tag="bst")
                    nc.vector.bn_stats(out=stats[0:ck], in_=pu[0:ck])
                    mv = work.tile([128, nc.vector.BN_AGGR_DIM], F32, tag="mv")
                    nc.vector.bn_aggr(out=mv[0:ck], in_=stats[0:ck])
                    rstd = work.tile([128, 1], F32, tag="rstd")
                    nc.scalar.activation(out=rstd[0:ck], in_=mv[0:ck, 1:2], func=AF.Sqrt,
                                         bias=eps_t[0:ck])
                    nc.vector.reciprocal(out=rstd[0:ck], in_=rstd[0:ck])
                    xn = work.tile([128, 64], F32, tag="xn")
                    nc.vector.tensor_scalar(out=xn[0:ck], in0=pu[0:ck], scalar1=mv[0:ck, 0:1],
                                            scalar2=rstd[0:ck], op0=ALU.subtract, op1=ALU.mult)
                    nc.vector.tensor_mul(out=xn[0:ck], in0=xn[0:ck], in1=lng[l][0:ck])
                    nc.vector.tensor_add(out=xn[0:ck], in0=xn[0:ck], in1=lnb[l][0:ck])
                    hnew = work.tile([128, 64], BF16, tag="hnew")
                    nc.scalar.activation(out=hnew[0:ck], in_=xn[0:ck], func=AF.Relu)
                    pt = ps_t.tile([64, 128], BF16, tag="pt")
                    nc.tensor.transpose(pt[:, 0:ck], hnew[0:ck], WBS("identB", slice(0, ck))[:, 0:ck])
                    nc.vector.tensor_copy(out=hT_next[:, i0:i0 + ck], in_=pt[:, 0:ck])
                hT_cur = hT_next
                if dbg:
                    nc.gpsimd.dma_start(out=dbg[f"dbg_h{l + 1}"], in_=hT_next)

        # ---------------- MLP prep
        hT2 = consts.tile([128, 200], BF16, tag="hT2x")
        nc.gpsimd.dma_start(out=hT2[0:64], in_=hT_cur[0:64])
        nc.gpsimd.dma_start(out=hT2[64:128], in_=hT_cur[0:64])
        Cb = consts.tile([64, 200], F32, tag="Cb")
        with tc.tile_pool(name="ps_prep", bufs=1, space="PSUM") as ps_prep:
            pC = ps_prep.tile([64, 200], F32)
            nc.tensor.matmul(pC, WBS("W1hjT", slice(0, 64)), hT_cur[0:64], start=True, stop=True)
            nc.scalar.activation(out=Cb, in_=pC, func=AF.Identity, bias=WFS("b1col", slice(0, 64)))
        CbS = consts.tile([128, 100], F32, tag="CbS")
        ev = bass.AP(tensor=Cb.tensor, offset=Cb.offset, ap=[Cb.ap[0], [2, 100]])
        od = bass.AP(tensor=Cb.tensor, offset=Cb.offset + 1, ap=[Cb.ap[0], [2, 100]])
        nc.gpsimd.dma_start(out=CbS[0:64], in_=ev)
        nc.gpsimd.dma_start(out=CbS[64:128], in_=od)
        if dbg:
            nc.gpsimd.dma_start(out=dbg["dbg_cb"], in_=Cb)

        # ---------------- MLP main loop
        with tc.tile_pool(name="ps1", bufs=3, space="PSUM") as ps1, \
             tc.tile_pool(name="ps2", bufs=2, space="PSUM") as ps2, \
             tc.tile_pool(name="ps3", bufs=1, space="PSUM") as ps3:
            p3 = ps3.tile([128, 200], F32)
            nc.vector.memset(p3, 0.0)
            for g in range(NGROUP):
                ntile = 2 if g < 12 else 1
                stage = work.tile([128, 200], F32, tag="stage")
                for half in range(ntile):
                    m = 2 * g + half
                    esl = edge_mlp[:, 200 * m:200 * m + 200]
                    p2 = None
                    for p in range(4):
                        pair = 4 * m + p
                        p1 = ps1.tile([128, 200], F32, tag="p1")
                        nc.tensor.matmul(p1, WBS("lhsT_e", slice(32 * p, 32 * p + 10)),
                                         esl[32 * p:32 * p + 10, :], start=True, stop=False,
                                         tile_position=(32 * p, 0))
                        nc.tensor.matmul(p1, WBS("lhsT_W1hi"), hT2, start=False, stop=True)
                        z1 = work.tile([128, 200], BF16, tag="z1")
                        bias = CbS[:, pair:pair + 1]
                        if pair % 2 == 0:
                            nc.scalar.activation(out=z1, in_=p1, func=AF.Relu, bias=bias)
                        else:
                            nc.vector.tensor_scalar(out=z1, in0=p1, scalar1=bias, scalar2=0.0,
                                                    op0=ALU.add, op1=ALU.max)
                        if p % 2 == 0:
                            p2 = ps2.tile([128, 200], F32, tag="p2")
                        nc.tensor.matmul(p2[64 * (p % 2):64 * (p % 2) + 64, :],
                                         WBS("lhsT_W2"), z1, start=True, stop=True,
                                         skip_group_check=True)
                        if p % 2 == 1:
                            z2r = work.tile([128, 200], BF16, tag="z2")
                            nc.vector.tensor_scalar(out=z2r, in0=p2, scalar1=WFS("b2x4"),
                                                    scalar2=0.0, op0=ALU.add, op1=ALU.max)
                            q = 2 * half + p // 2
                            nc.tensor.matmul(p3[32 * q:32 * q + 4, :], WBS("lhsT_W3"), z2r,
                                             start=True, stop=True, skip_group_check=True,
                                             tile_position=(0, 32 * q))
                nc.scalar.activation(out=stage, in_=p3, func=AF.Sigmoid,
                                     bias=WFS("b3col"))
                for q in range(4 if g < 12 else 2):
                    nc.gpsimd.dma_start(out=d_out[16 * g + 4 * q:16 * g + 4 * q + 4, :],
                                        in_=stage[32 * q:32 * q + 4, :])


# ---------------------------------------------------------------- entry points
_CACHE = {}


def _get_nc(debug=False):
    key = bool(debug)
    if key not in _CACHE:
        _CACHE[key] = _build(debug)
    return _CACHE[key]


def _make_in_maps(inputs):
    wb, wf = _pack_weights(inputs)
    maps = []
    for b in range(B):
        m = _pack_core(inputs, b)
        m["wb16"] = wb
        m["wf32"] = wf
        maps.append(m)
    return maps


def _run(inputs, trace=False, debug=False):
    nc = _get_nc(debug)
    in_maps = _make_in_maps(inputs)
    res = bass_utils.run_bass_kernel_spmd(nc, in_maps, core_ids=list(range(B)), trace=trace)
    outs = np.stack([res.results[b]["outT"].T for b in range(B)], axis=0).astype(np.float32)
    return outs, res


def kernel(**inputs):
    inputs = {k: np.asarray(v) for k, v in inputs.items()}
    outs, _ = _run(inputs, trace=False)
    return outs
